# revision 4
# baseline (speedup 1.0000x reference)
"""AttnDecoderRNN single-step decoder on 8 Trainium2 NeuronCores (Bass/Tile).

Sharding (8 cores):
  - LSTM hidden-sharded: core k owns hidden units [128k, 128k+128). Gate weights are
    pre-arranged host-side as transposed gate-major blocks; gates computed as
    gatesT [512, 64] on PE so biases are per-partition. AllGather -> hT_full [1024, 64].
  - Attention batch-sharded: core k owns batches [8k, 8k+8) (64 MB encoder shard).
    scores via fused DVE scalar_tensor_tensor (mult+reduce in one pass),
    context via PE matmul (lhsT=scores chunk [128,1], rhs=E tile) accumulating in PSUM.
    Encoder tiles are read from HBM exactly once. AllGather context -> [64, 1024].
  - FC vocab-sharded: core k owns vocab rows [4000k, 4000k+4000) with fc_W.T staged
    host-side; fc_b added via a K=1 matmul row. Log-softmax uses local max/sumexp then
    one AllGather of stats; each core emits its logp shard [64, 4000].
Host reassembles: logp concat on vocab axis; h/c from per-core [128, 64] shards.
"""
import numpy as np

import concourse.bass as bass
import concourse.bacc as bacc
import concourse.tile as tile
import concourse.mybir as mybir
from concourse import bass_utils
from concourse.bass import ds
from concourse.masks import make_identity

# problem dims (hardcoded per contract)
V, E, H = 32000, 512, 1024
N, S = 64, 2048
P = 128
NCORES = 8
NB = N // NCORES            # 8 batches per core
HS = H // NCORES            # 128 hidden units per core
VS = V // NCORES            # 4000 vocab rows per core
G4 = 4 * HS                 # 512 gate rows per core
SCH = S // P                # 16 seq chunks per batch
SCH_PER_DMA = 2             # seq chunks per encoder DMA (1 MB transfers)
NCH = 16                    # fc vocab chunks per core
NW = VS // NCH              # 500 vocab per fc chunk
KCH = (2 * H) // P          # 16 contraction chunks for fc
ECH = E // P                # 4
HCH = H // P                # 8

F32 = mybir.dt.float32
I32 = mybir.dt.int32
RG = [list(range(NCORES))]

_TRACE = False
_LAST_RESULTS = None
_NC_CACHE = None


def _build_nc():
    nc = bacc.Bacc("TRN2", target_bir_lowering=False, debug=False,
                   enable_asserts=True, num_devices=NCORES)
    AF = mybir.ActivationFunctionType

    # ---- per-core inputs ----
    d_idx = nc.dram_tensor("idx", [N, 1], I32, kind="ExternalInput")
    d_bidx = nc.dram_tensor("bidx", [P, NB], I32, kind="ExternalInput")
    d_h0T = nc.dram_tensor("h0T", [H, N], F32, kind="ExternalInput")
    d_c0T = nc.dram_tensor("c0T", [HS, N], F32, kind="ExternalInput")
    d_enc = nc.dram_tensor("enc", [NB, S, H], F32, kind="ExternalInput")
    d_emb = nc.dram_tensor("embW", [V, E], F32, kind="ExternalInput")
    d_wihT = nc.dram_tensor("wihT", [E, G4], F32, kind="ExternalInput")
    d_whhT = nc.dram_tensor("whhT", [H, G4], F32, kind="ExternalInput")
    d_bih = nc.dram_tensor("bihS", [G4, 1], F32, kind="ExternalInput")
    d_bhh = nc.dram_tensor("bhhS", [G4, 1], F32, kind="ExternalInput")
    d_fcWT = nc.dram_tensor("fcWT", [2 * H, VS], F32, kind="ExternalInput")
    d_fcb = nc.dram_tensor("fcb", [1, VS], F32, kind="ExternalInput")

    # ---- per-core outputs ----
    o_logp = nc.dram_tensor("logp", [N, VS], F32, kind="ExternalOutput")
    o_hT = nc.dram_tensor("hT_out", [HS, N], F32, kind="ExternalOutput")
    o_cT = nc.dram_tensor("cT_out", [HS, N], F32, kind="ExternalOutput")

    with tile.TileContext(nc) as tc:
        with (
            tc.tile_pool(name="const", bufs=1) as cpool,
            tc.tile_pool(name="lstm", bufs=1) as lpool,
            tc.tile_pool(name="enc", bufs=4) as epool,
            tc.tile_pool(name="hb", bufs=2) as hbpool,
            tc.tile_pool(name="sc", bufs=6) as scpool,
            tc.tile_pool(name="scr", bufs=2) as scrpool,
            tc.tile_pool(name="fcw", bufs=2) as fpool,
            tc.tile_pool(name="big", bufs=1) as bpool,
            tc.tile_pool(name="small", bufs=2) as spool,
            tc.tile_pool(name="psT", bufs=1, space="PSUM") as psT,   # transposes
            tc.tile_pool(name="dram", bufs=1, space="DRAM") as dpool,
        ):
            ident = cpool.tile([P, P], F32)
            make_identity(nc, ident[:])
            ones_row = cpool.tile([1, P], F32)
            nc.gpsimd.memset(ones_row[:], 1.0)

            phase1 = tc.tile_pool(name="psA", bufs=1, space="PSUM")
            psA = phase1.__enter__()
            phase1b = tc.tile_pool(name="psTB", bufs=1, space="PSUM")
            psTB = phase1b.__enter__()

            # ================= LSTM (hidden-sharded) =================
            idx_sb = lpool.tile([N, 1], I32)
            nc.sync.dma_start(idx_sb[:], d_idx[:, :])
            x_sb = lpool.tile([N, E], F32)
            nc.gpsimd.indirect_dma_start(
                out=x_sb[:], out_offset=None,
                in_=d_emb[:, :],
                in_offset=bass.IndirectOffsetOnAxis(ap=idx_sb[:, :1], axis=0),
            )
            # xT [E, N] = 4 PE transposes of x chunks
            xT_sb = lpool.tile([P, ECH, N], F32)
            for ce in range(ECH):
                ptp = psT.tile([P, N], F32, tag="tp")
                nc.tensor.transpose(ptp[:], x_sb[:, ds(ce * P, P)], ident[:N, :N])
                nc.scalar.copy(xT_sb[:, ce, :], ptp[:])

            h0T_sb = lpool.tile([P, HCH, N], F32)
            nc.sync.dma_start(h0T_sb[:], d_h0T.ap().rearrange("(c p) n -> p c n", p=P))
            c0T_sb = lpool.tile([HS, N], F32)
            nc.sync.dma_start(c0T_sb[:], d_c0T[:, :])
            wihT_sb = lpool.tile([P, ECH, G4], F32)
            nc.sync.dma_start(wihT_sb[:], d_wihT.ap().rearrange("(c p) g -> p c g", p=P))
            whhT_sb = lpool.tile([P, HCH, G4], F32)
            nc.sync.dma_start(whhT_sb[:], d_whhT.ap().rearrange("(c p) g -> p c g", p=P))
            bih_sb = lpool.tile([P, 4], F32)
            nc.sync.dma_start(bih_sb[:], d_bih.ap().rearrange("(g p) o -> p (g o)", p=P))
            bhh_sb = lpool.tile([P, 4], F32)
            nc.sync.dma_start(bhh_sb[:], d_bhh.ap().rearrange("(g p) o -> p (g o)", p=P))
            badd = lpool.tile([P, 4], F32)
            nc.vector.tensor_add(badd[:], bih_sb[:], bhh_sb[:])

            # gatesT: 4 gate tiles [128, 64] (i, f, g, o)
            gate_ps = []
            for cg in range(4):
                pg = psA.tile([P, N], F32, tag=f"g{cg}")
                for ce in range(ECH):
                    nc.tensor.matmul(pg[:], wihT_sb[:, ce, ds(cg * P, P)],
                                     xT_sb[:, ce, :], start=(ce == 0), stop=False)
                for ch in range(HCH):
                    nc.tensor.matmul(pg[:], whhT_sb[:, ch, ds(cg * P, P)],
                                     h0T_sb[:, ch, :], start=False, stop=(ch == HCH - 1))
                gate_ps.append(pg)

            a_i = lpool.tile([P, N], F32)
            a_f = lpool.tile([P, N], F32)
            a_g = lpool.tile([P, N], F32)
            a_o = lpool.tile([P, N], F32)
            nc.scalar.activation(a_i[:], gate_ps[0][:], AF.Sigmoid, bias=badd[:, 0:1])
            nc.scalar.activation(a_f[:], gate_ps[1][:], AF.Sigmoid, bias=badd[:, 1:2])
            nc.scalar.activation(a_g[:], gate_ps[2][:], AF.Tanh, bias=badd[:, 2:3])
            nc.scalar.activation(a_o[:], gate_ps[3][:], AF.Sigmoid, bias=badd[:, 3:4])

            t0 = lpool.tile([P, N], F32)
            nc.vector.tensor_mul(t0[:], a_f[:], c0T_sb[:])
            t1 = lpool.tile([P, N], F32)
            nc.vector.tensor_mul(t1[:], a_i[:], a_g[:])
            cT_sb = lpool.tile([P, N], F32)
            nc.vector.tensor_add(cT_sb[:], t0[:], t1[:])
            th_sb = lpool.tile([P, N], F32)
            nc.scalar.activation(th_sb[:], cT_sb[:], AF.Tanh)
            hT_sb = lpool.tile([P, N], F32)
            nc.vector.tensor_mul(hT_sb[:], a_o[:], th_sb[:])

            nc.sync.dma_start(o_cT[:, :], cT_sb[:])
            nc.sync.dma_start(o_hT[:, :], hT_sb[:])

            # ============ AllGather h: [128, 64] -> [1024, 64] ============
            h_bounce = dpool.tile([HS, N], F32)
            hTf_dram = dpool.tile([H, N], F32)
            nc.sync.dma_start(h_bounce[:], hT_sb[:])
            nc.gpsimd.collective_compute(
                "AllGather", mybir.AluOpType.bypass, replica_groups=RG,
                ins=[h_bounce.opt()], outs=[hTf_dram.opt()],
            )
            hTf_sb = bpool.tile([P, HCH, N], F32)
            nc.sync.dma_start(hTf_sb[:], hTf_dram[:].rearrange("(c p) n -> p c n", p=P))

            # h rows [batch, hidden]: transpose all 64 batches, store to DRAM
            ptall = psTB.tile([N, HCH, P], F32, tag="tpbig")
            for ch in range(HCH):
                nc.tensor.transpose(ptall[:, ch, :], hTf_sb[:, ch, :], ident[:, :])
            hrall_sb = bpool.tile([N, HCH, P], F32)
            nc.scalar.copy(hrall_sb[:], ptall[:])
            hrall_dram = dpool.tile([N, H], F32)
            nc.sync.dma_start(hrall_dram[:], hrall_sb[:].rearrange("n c p -> n (c p)"))

            phase1b.__exit__(None, None, None)
            phase1.__exit__(None, None, None)
            phase2c = tc.tile_pool(name="psC", bufs=2, space="PSUM")
            psC = phase2c.__enter__()
            phase2l = tc.tile_pool(name="psL", bufs=2, space="PSUM")
            psL = phase2l.__enter__()

            # ================= attention (batch-sharded) =================
            # broadcast h_j across all 128 partitions via replicated-index gather
            bidx_sb = lpool.tile([P, NB], I32)
            nc.sync.dma_start(bidx_sb[:], d_bidx[:, :])
            ctx_bounce = dpool.tile([NB, H], F32)
            ctxf_dram = dpool.tile([N, H], F32)
            for j in range(NB):
                hb_j = hbpool.tile([P, H], F32, tag="hb")
                nc.gpsimd.indirect_dma_start(
                    out=hb_j[:], out_offset=None,
                    in_=hrall_dram[:],
                    in_offset=bass.IndirectOffsetOnAxis(ap=bidx_sb[:, j:j + 1], axis=0),
                )
                pc = psC.tile([1, H], F32, tag="ctx")
                for tt in range(SCH // SCH_PER_DMA):
                    et = epool.tile([P, SCH_PER_DMA, H], F32, tag="enc")
                    nc.sync.dma_start(
                        et[:],
                        d_enc.ap()[j, ds(tt * SCH_PER_DMA * P, SCH_PER_DMA * P), :]
                        .rearrange("(a p) h -> p a h", p=P),
                    )
                    for a in range(SCH_PER_DMA):
                        t = tt * SCH_PER_DMA + a
                        scr = scrpool.tile([P, H], F32, tag="scr")
                        sc = scpool.tile([P, 1], F32, tag="sc")
                        nc.vector.scalar_tensor_tensor(
                            out=scr[:], in0=et[:, a, :], scalar=1.0, in1=hb_j[:],
                            op0=mybir.AluOpType.mult, op1=mybir.AluOpType.mult,
                            accum_out=sc[:],
                        )
                        nc.tensor.matmul(pc[:, 0:512], sc[:], et[:, a, 0:512],
                                         start=(t == 0), stop=(t == SCH - 1))
                        nc.tensor.matmul(pc[:, 512:1024], sc[:], et[:, a, 512:1024],
                                         start=(t == 0), stop=(t == SCH - 1))
                ctxr = scrpool.tile([1, H], F32, tag="ctxr")
                nc.scalar.copy(ctxr[:], pc[:])
                nc.sync.dma_start(ctx_bounce[j:j + 1, :], ctxr[:])

            # ============ AllGather context: [8, 1024] -> [64, 1024] ============
            nc.gpsimd.collective_compute(
                "AllGather", mybir.AluOpType.bypass, replica_groups=RG,
                ins=[ctx_bounce.opt()], outs=[ctxf_dram.opt()],
            )
            ctxf_sb = bpool.tile([N, H], F32)
            nc.sync.dma_start(ctxf_sb[:], ctxf_dram[:])
            ctxT_sb = bpool.tile([P, HCH, N], F32)
            for ch in range(HCH):
                ptc = psT.tile([P, N], F32, tag="tp")
                nc.tensor.transpose(ptc[:], ctxf_sb[:, ds(ch * P, P)], ident[:N, :N])
                nc.scalar.copy(ctxT_sb[:, ch, :], ptc[:])

            # ============ FC (vocab-sharded) + local softmax stats ============
            fcb_sb = cpool.tile([1, VS], F32)
            nc.sync.dma_start(fcb_sb[:], d_fcb[:, :])
            logits_sb = bpool.tile([N, VS], F32)
            mpart = spool.tile([N, NCH], F32, tag="mpart")
            for nch in range(NCH):
                fw = fpool.tile([P, KCH, NW], F32, tag="fw")
                nc.scalar.dma_start(
                    fw[:],
                    d_fcWT.ap().rearrange("(c p) v -> p c v", p=P)[:, :, ds(nch * NW, NW)],
                )
                pl = psL.tile([N, NW], F32, tag="lg")
                for kc in range(KCH):
                    lhsT = hTf_sb[:, kc, :] if kc < HCH else ctxT_sb[:, kc - HCH, :]
                    nc.tensor.matmul(pl[:], lhsT, fw[:, kc, :],
                                     start=(kc == 0), stop=False)
                nc.tensor.matmul(pl[:], ones_row[0:1, :N], fcb_sb[0:1, ds(nch * NW, NW)],
                                 start=False, stop=True)
                nc.scalar.copy(logits_sb[:, ds(nch * NW, NW)], pl[:])
                nc.vector.tensor_reduce(mpart[:, nch:nch + 1], pl[:],
                                        axis=mybir.AxisListType.X, op=mybir.AluOpType.max)

            m_local = spool.tile([N, 1], F32, tag="m_local")
            nc.vector.tensor_reduce(m_local[:], mpart[:], axis=mybir.AxisListType.X,
                                    op=mybir.AluOpType.max)
            neg_m = spool.tile([N, 1], F32, tag="neg_m")
            nc.vector.tensor_scalar_mul(neg_m[:], m_local[:], -1.0)
            spart = spool.tile([N, NCH], F32, tag="spart")
            for nch in range(NCH):
                scr_e = scrpool.tile([N, NW], F32, tag="scr_e")
                nc.scalar.activation(scr_e[:], logits_sb[:, ds(nch * NW, NW)], AF.Exp,
                                     bias=neg_m[:], accum_out=spart[:, nch:nch + 1])
            s_local = spool.tile([N, 1], F32, tag="s_local")
            nc.vector.tensor_reduce(s_local[:], spart[:], axis=mybir.AxisListType.X,
                                    op=mybir.AluOpType.add)

            # ============ AllGather stats: [2, 64] -> [16, 64] ============
            stats_sb = spool.tile([N, 2], F32, tag="stats")
            nc.vector.tensor_copy(stats_sb[:, 0:1], m_local[:])
            nc.vector.tensor_copy(stats_sb[:, 1:2], s_local[:])
            pst = psT.tile([2, N], F32, tag="tp")
            nc.tensor.transpose(pst[:], stats_sb[:], ident[:N, :N])
            statsT_sb = spool.tile([2, N], F32, tag="statsT")
            nc.scalar.copy(statsT_sb[:], pst[:])
            st_bounce = dpool.tile([2, N], F32)
            stAG_dram = dpool.tile([2 * NCORES, N], F32)
            nc.sync.dma_start(st_bounce[:], statsT_sb[:])
            nc.gpsimd.collective_compute(
                "AllGather", mybir.AluOpType.bypass, replica_groups=RG,
                ins=[st_bounce.opt()], outs=[stAG_dram.opt()],
            )
            stAG_sb = spool.tile([2 * NCORES, N], F32, tag="stAG")
            nc.sync.dma_start(stAG_sb[:], stAG_dram[:])
            pstt = psT.tile([N, 2 * NCORES], F32, tag="tp")
            nc.tensor.transpose(pstt[:], stAG_sb[:], ident[:2 * NCORES, :2 * NCORES])
            mall = spool.tile([N, NCORES], F32, tag="mall")
            sall = spool.tile([N, NCORES], F32, tag="sall")
            pstt_v = pstt[:].rearrange("n (r two) -> n r two", two=2)
            nc.scalar.copy(mall[:], pstt_v[:, :, 0:1].rearrange("n r o -> n (r o)"))
            nc.scalar.copy(sall[:], pstt_v[:, :, 1:2].rearrange("n r o -> n (r o)"))

            gmax = spool.tile([N, 1], F32, tag="gmax")
            nc.vector.tensor_reduce(gmax[:], mall[:], axis=mybir.AxisListType.X,
                                    op=mybir.AluOpType.max)
            neg_gmax = spool.tile([N, 1], F32, tag="neg_gmax")
            nc.vector.tensor_scalar_mul(neg_gmax[:], gmax[:], -1.0)
            corr = spool.tile([N, NCORES], F32, tag="corr")
            nc.scalar.activation(corr[:], mall[:], AF.Exp, bias=neg_gmax[:])
            scr_c = spool.tile([N, NCORES], F32, tag="scr_c")
            gsum = spool.tile([N, 1], F32, tag="gsum")
            nc.vector.scalar_tensor_tensor(
                out=scr_c[:], in0=corr[:], scalar=1.0, in1=sall[:],
                op0=mybir.AluOpType.mult, op1=mybir.AluOpType.mult, accum_out=gsum[:],
            )
            lng = spool.tile([N, 1], F32, tag="lng")
            nc.scalar.activation(lng[:], gsum[:], AF.Ln)
            off = spool.tile([N, 1], F32, tag="off")
            nc.vector.tensor_add(off[:], gmax[:], lng[:])
            neg_off = spool.tile([N, 1], F32, tag="neg_off")
            nc.vector.tensor_scalar_mul(neg_off[:], off[:], -1.0)

            # final: logp = logits - (gmax + ln(gsum))
            for nch in range(NCH):
                outc = scrpool.tile([N, NW], F32, tag="outc")
                nc.scalar.activation(outc[:], logits_sb[:, ds(nch * NW, NW)],
                                     AF.Identity, bias=neg_off[:])
                nc.sync.dma_start(o_logp[:, ds(nch * NW, NW)], outc[:])

            phase2l.__exit__(None, None, None)
            phase2c.__exit__(None, None, None)

    nc.compile()
    return nc


def _get_nc():
    global _NC_CACHE
    if _NC_CACHE is None:
        _NC_CACHE = _build_nc()
    return _NC_CACHE


def kernel(prev_outputs, prev_hn, prev_cn, encoder_hidden_states,
           emb_W, w_ih, w_hh, b_ih, b_hh, fc_W, fc_b):
    global _LAST_RESULTS
    prev_outputs = np.asarray(prev_outputs)
    prev_hn = np.asarray(prev_hn, dtype=np.float32)
    prev_cn = np.asarray(prev_cn, dtype=np.float32)
    encoder_hidden_states = np.asarray(encoder_hidden_states, dtype=np.float32)
    emb_W = np.asarray(emb_W, dtype=np.float32)
    w_ih = np.asarray(w_ih, dtype=np.float32)
    w_hh = np.asarray(w_hh, dtype=np.float32)
    b_ih = np.asarray(b_ih, dtype=np.float32)
    b_hh = np.asarray(b_hh, dtype=np.float32)
    fc_W = np.asarray(fc_W, dtype=np.float32)
    fc_b = np.asarray(fc_b, dtype=np.float32)

    nc = _get_nc()

    idx = np.ascontiguousarray(prev_outputs.reshape(N, 1).astype(np.int32))
    h0T = np.ascontiguousarray(prev_hn[0].T)                     # [H, N]
    in_maps = []
    for k in range(NCORES):
        hs = slice(k * HS, (k + 1) * HS)
        vs = slice(k * VS, (k + 1) * VS)
        wih_rows = np.concatenate([w_ih[g * H + k * HS:g * H + (k + 1) * HS] for g in range(4)], axis=0)
        whh_rows = np.concatenate([w_hh[g * H + k * HS:g * H + (k + 1) * HS] for g in range(4)], axis=0)
        bih_rows = np.concatenate([b_ih[g * H + k * HS:g * H + (k + 1) * HS] for g in range(4)], axis=0)
        bhh_rows = np.concatenate([b_hh[g * H + k * HS:g * H + (k + 1) * HS] for g in range(4)], axis=0)
        bidx = np.tile(np.arange(k * NB, (k + 1) * NB, dtype=np.int32)[None, :], (P, 1))
        in_maps.append({
            "idx": idx,
            "bidx": np.ascontiguousarray(bidx),
            "h0T": h0T,
            "c0T": np.ascontiguousarray(prev_cn[0][:, hs].T),    # [HS, N]
            "enc": np.ascontiguousarray(encoder_hidden_states[k * NB:(k + 1) * NB]),
            "embW": emb_W,
            "wihT": np.ascontiguousarray(wih_rows.T),            # [E, 4*HS]
            "whhT": np.ascontiguousarray(whh_rows.T),            # [H, 4*HS]
            "bihS": np.ascontiguousarray(bih_rows.reshape(G4, 1)),
            "bhhS": np.ascontiguousarray(bhh_rows.reshape(G4, 1)),
            "fcWT": np.ascontiguousarray(fc_W[vs].T),            # [2H, VS]
            "fcb": np.ascontiguousarray(fc_b[vs].reshape(1, VS)),
        })

    res = bass_utils.run_bass_kernel_spmd(nc, in_maps, core_ids=list(range(NCORES)),
                                          trace=_TRACE)
    _LAST_RESULTS = res

    logp = np.concatenate([res.results[k]["logp"] for k in range(NCORES)], axis=1)
    h = np.concatenate([res.results[k]["hT_out"].T for k in range(NCORES)], axis=1)[None]
    c = np.concatenate([res.results[k]["cT_out"].T for k in range(NCORES)], axis=1)[None]
    return logp, h, c
